# revision 1
# baseline (speedup 1.0000x reference)
"""Trainium2 Bass kernel for nn_Decoder (hypernet CA decoder).

Self-contained: hardcodes all shapes. Strategy:
  - 8-way data parallel over batch B=8 (1 sample per NeuronCore).
  - Hypernet GEMV column-sharded 8x (with a host-side column permutation so
    each core's shard unpacks into transposed weight tiles), then AllToAll so
    core b receives sample b's full parameter vector.
  - State kept channels-major, duplicated across partitions as [x; -x]
    (padded [H+2, W+2] image per partition) so all vector ops stay
    lane-aligned and sobel-y can be formed on the upper partitions.
  - Instance norm folded into the per-sample 1x1-conv weights (per-channel
    scale/bias), sobel 1/8 and sy sign prebaked into the weight tiles.
  - Dyna block: z/g output channels duplicated to M=128 (free in matmul) so
    gating + the leak-update run 128 partitions wide.
"""

from contextlib import ExitStack

import numpy as np

import concourse.bacc as bacc
import concourse.bass as bass
import concourse.tile as tile
from concourse import mybir
from concourse.bass_utils import run_bass_kernel_spmd

AF = mybir.ActivationFunctionType
ALU = mybir.AluOpType
DT = mybir.dt.float32

N_CORES = 8
B, LAT, NF, IMG = 8, 512, 64, 128
FIN, FH, FOUT = 3 * NF, NF, 2 * NF
P_SIZES = [FH * FIN, FH, FOUT * FH, FOUT, FOUT * FIN, FOUT]
P_OFFS = np.cumsum([0] + P_SIZES)
P_TOT = int(P_OFFS[-1])         # 45376
CHUNK = P_TOT // N_CORES        # 5672
EPS = 1e-5

_g1 = np.exp(-0.5 * np.array([-1.0, 0.0, 1.0]) ** 2)
_gn = _g1 / _g1.sum()
G0, G1 = float(np.float32(_gn[0])), float(np.float32(_gn[1]))
# composed bilinear-up2 + gauss-blur 3-tap filters (per output parity)
A_E = 0.75 * G0 + 0.25 * G1
B_E = G0 + 0.75 * G1
C_E = 0.25 * G0

RES_PLAN = [(16, 4), (32, 4), (64, 4), (128, 4)]
STRIP_ROWS = {16: 16, 32: 32, 64: 32, 128: 16}
TILE_ROWS = {16: 16, 32: 16, 64: 8, 128: 4}

# ---- permuted p-vector layout (see build_perm) ----
Q_LEN = [4096, 8192, 64, 4096, 4096, 4096, 4096, 8192, 8192, 128, 128]
Q_OFF = np.cumsum([0] + Q_LEN)
(Q_W1X, Q_W1Z, Q_B1, Q_W2Z, Q_W2G, Q_WSXZ, Q_WSXG, Q_WSZZ, Q_WSZG,
 Q_B2, Q_BS) = [int(o) for o in Q_OFF[:-1]]


def build_perm():
    """Permutation of hyper_w columns so p_lin arrives pre-transposed."""
    idx = []
    o = np.arange(FH)
    # w1xT [i 0:64, o 0:64]
    for i in range(NF):
        idx.append(P_OFFS[0] + o * FIN + i)
    # w1zT [i 64:192, o 0:64]
    for i in range(NF, FIN):
        idx.append(P_OFFS[0] + o * FIN + i)
    # b1
    idx.append(P_OFFS[1] + np.arange(FH))
    o64 = np.arange(NF)
    # w2T_z [i 0:64, o 0:64] ; w2T_g [i 0:64, o 64:128]
    for i in range(FH):
        idx.append(P_OFFS[2] + o64 * FH + i)
    for i in range(FH):
        idx.append(P_OFFS[2] + (NF + o64) * FH + i)
    # wsxT_z / wsxT_g [i 0:64]
    for i in range(NF):
        idx.append(P_OFFS[4] + o64 * FIN + i)
    for i in range(NF):
        idx.append(P_OFFS[4] + (NF + o64) * FIN + i)
    # wszT_z / wszT_g [i 64:192]
    for i in range(NF, FIN):
        idx.append(P_OFFS[4] + o64 * FIN + i)
    for i in range(NF, FIN):
        idx.append(P_OFFS[4] + (NF + o64) * FIN + i)
    # b2, bs
    idx.append(P_OFFS[3] + np.arange(FOUT))
    idx.append(P_OFFS[5] + np.arange(FOUT))
    perm = np.concatenate(idx)
    assert perm.shape == (P_TOT,)
    return perm


def seed_branch_consts(seed):
    """Replicate jax-cpu inst_norm(broadcast(seed)) exactly (sequential sums)."""
    sv = np.asarray(seed, np.float32).reshape(NF)
    out = np.zeros(NF, np.float32)
    for c in range(NF):
        acc = np.float32(0.0)
        for _ in range(256):
            acc = np.float32(acc + sv[c])
        m = np.float32(acc / np.float32(256.0))
        d = np.float32(sv[c] - m)
        acc2 = np.float32(0.0)
        d2 = np.float32(d * d)
        for _ in range(256):
            acc2 = np.float32(acc2 + d2)
        v = np.float32(acc2 / np.float32(256.0))
        out[c] = np.float32(d * np.float32(1.0 / np.sqrt(np.float32(v) + np.float32(EPS))))
    return out.reshape(NF, 1)


def _axis3tap(nc, out3, src_lo, src_c, src_hi, c_lo, c_c, c_hi):
    """out = c_lo*src_lo + c_c*src_c + c_hi*src_hi (3 ops, exact fp32)."""
    nc.vector.tensor_scalar_mul(out=out3, in0=src_lo, scalar1=c_lo)
    nc.vector.scalar_tensor_tensor(out=out3, in0=src_c, scalar=c_c, in1=out3,
                                   op0=ALU.mult, op1=ALU.add)
    nc.vector.scalar_tensor_tensor(out=out3, in0=src_hi, scalar=c_hi, in1=out3,
                                   op0=ALU.mult, op1=ALU.add)


def _axis2tap(nc, out2, src_a, src_b, c_a, c_b):
    nc.vector.tensor_scalar_mul(out=out2, in0=src_a, scalar1=c_a)
    nc.vector.scalar_tensor_tensor(out=out2, in0=src_b, scalar=c_b, in1=out2,
                                   op0=ALU.mult, op1=ALU.add)


def build_nc():
    nc = bacc.Bacc("TRN2", target_bir_lowering=False, debug=False,
                   num_devices=N_CORES)

    latT = nc.declare_dram_parameter("latT", [LAT, B], DT, isOutput=False)
    hw = nc.declare_dram_parameter("hw", [LAT, CHUNK], DT, isOutput=False)
    hb = nc.declare_dram_parameter("hb", [1, CHUNK], DT, isOutput=False)
    ca = nc.declare_dram_parameter("ca", [NF, 256], DT, isOutput=False)
    lfv = nc.declare_dram_parameter("lfv", [128, 1], DT, isOutput=False)
    seedc = nc.declare_dram_parameter("seedc", [NF, 1], DT, isOutput=False)
    r1T = nc.declare_dram_parameter("r1T", [NF, NF], DT, isOutput=False)
    r1b = nc.declare_dram_parameter("r1b", [NF, 1], DT, isOutput=False)
    r2T = nc.declare_dram_parameter("r2T", [NF, NF], DT, isOutput=False)
    r2b = nc.declare_dram_parameter("r2b", [NF, 1], DT, isOutput=False)
    ocT = nc.declare_dram_parameter("ocT", [NF, 3], DT, isOutput=False)
    ocb = nc.declare_dram_parameter("ocb", [3, 1], DT, isOutput=False)
    y_clip = nc.declare_dram_parameter("y_clip", [3, IMG * IMG], DT, isOutput=True)
    y_raw = nc.declare_dram_parameter("y_raw", [3, IMG * IMG], DT, isOutput=True)

    with tile.TileContext(nc) as tc, ExitStack() as ctx:
        with tc.tile_pool(name="dramp", bufs=1, space="DRAM") as dp:
            a2a_in = dp.tile([N_CORES, CHUNK], DT)
            a2a_out = dp.tile([N_CORES, CHUNK], DT)

            # ---------------- Phase 1: hypernet GEMV + AllToAll ------------
            with (
                tc.tile_pool(name="gemv_s", bufs=1) as gs,
                tc.tile_pool(name="gemv_w", bufs=3) as gw,
                tc.tile_pool(name="gemv_ps", bufs=2, space="PSUM") as gp,
            ):
                latT_s = gs.tile([128, 4, B], DT)
                nc.sync.dma_start(
                    out=latT_s[:], in_=latT.rearrange("(k p) b -> p k b", p=128))
                hb8 = gs.tile([B, CHUNK], DT)
                hb_ap = hb[0:1, :]
                hb_bcast = bass.AP(tensor=hb_ap.tensor, offset=hb_ap.offset,
                                   ap=[[0, B]] + list(hb_ap.ap)[1:])
                nc.sync.dma_start(out=hb8[:], in_=hb_bcast)
                p_sb = gs.tile([B, CHUNK], DT)
                hwv = hw.rearrange("(k p) n -> p k n", p=128)
                n_t = (CHUNK + 511) // 512
                for t in range(n_t):
                    n0 = t * 512
                    nn = min(CHUNK, n0 + 512) - n0
                    hw_s = gw.tile([128, 4, 512], DT, tag="hw")
                    nc.sync.dma_start(out=hw_s[:, :, :nn],
                                      in_=hwv[:, :, n0:n0 + nn])
                    ps = gp.tile([B, 512], DT, tag="ps")
                    for k in range(4):
                        nc.tensor.matmul(ps[:, :nn], latT_s[:, k, :],
                                         hw_s[:, k, :nn],
                                         start=(k == 0), stop=(k == 3))
                    nc.vector.tensor_add(out=p_sb[:, n0:n0 + nn],
                                         in0=ps[:, :nn],
                                         in1=hb8[:, n0:n0 + nn])
                nc.sync.dma_start(out=a2a_in.opt(), in_=p_sb[:])

            nc.gpsimd.collective_compute(
                "AllToAll", ALU.bypass,
                replica_groups=[list(range(N_CORES))],
                ins=[a2a_in.opt()], outs=[a2a_out.opt()])

            p_lin = a2a_out.opt().rearrange("r n -> (r n)")

            def seg(off, ln, inner):
                return p_lin[off:off + ln].rearrange("(i o) -> i o", o=inner)

            # ---------------- Phase 2: persistent tiles + unpack -----------
            sg = ctx.enter_context(tc.tile_pool(name="singles", bufs=1))
            xp2 = sg.tile([128, 130 * 130], DT)
            zA = sg.tile([128, IMG * IMG], DT)
            abP = sg.tile([128, 2112], DT)

            w1xT_r = sg.tile([NF, FH], DT)
            w1zT_r = sg.tile([128, FH], DT)
            w1xT_f = sg.tile([NF, FH], DT)
            w1zT_f = sg.tile([128, FH], DT)
            w2T_zd = sg.tile([NF, 128], DT)
            w2T_gd = sg.tile([NF, 128], DT)
            wsxT_zd_r = sg.tile([NF, 128], DT)
            wsxT_gd_r = sg.tile([NF, 128], DT)
            wszT_zd_r = sg.tile([128, 128], DT)
            wszT_gd_r = sg.tile([128, 128], DT)
            wsxT_zf = sg.tile([NF, 128], DT)
            wsxT_gf = sg.tile([NF, 128], DT)
            wszT_zf = sg.tile([128, 128], DT)
            wszT_gf = sg.tile([128, 128], DT)
            b1_r = sg.tile([NF, 1], DT)
            b1_f = sg.tile([NF, 1], DT)
            b2z_d = sg.tile([128, 1], DT)
            b2g_d = sg.tile([128, 1], DT)
            bsz_d = sg.tile([128, 1], DT)
            bsg_d = sg.tile([128, 1], DT)
            bzg_zd = sg.tile([128, 1], DT)
            bzg_gd = sg.tile([128, 1], DT)
            bz_f = sg.tile([128, 1], DT)
            bg_f = sg.tile([128, 1], DT)
            lf_s = sg.tile([128, 1], DT)
            pm_s = sg.tile([128, 1], DT)
            eps_s = sg.tile([128, 1], DT)
            seedc_s = sg.tile([NF, 1], DT)
            statz = sg.tile([128, 34, 6], DT)
            statx = sg.tile([128, 34, 6], DT)
            mx_c = sg.tile([128, 1], DT)
            vx_t = sg.tile([128, 1], DT)
            vx_u = sg.tile([128, 1], DT)
            mvz = sg.tile([128, 2], DT)
            mvx = sg.tile([128, 2], DT)
            tmp1 = sg.tile([128, 1], DT)
            tmp2 = sg.tile([128, 1], DT)
            rstdz = sg.tile([128, 1], DT)
            rstdx = sg.tile([128, 1], DT)
            r1T_s = sg.tile([NF, NF], DT)
            r1b_s = sg.tile([NF, 1], DT)
            r2T_s = sg.tile([NF, NF], DT)
            r2b_s = sg.tile([NF, 1], DT)
            ocT_s = sg.tile([NF, 3], DT)
            ocb_s = sg.tile([3, 1], DT)
            cas = sg.tile([NF, 18, 18], DT)
            tcol = sg.tile([NF, 16, 18], DT)

            dma = nc.sync.dma_start
            dma(out=w1xT_r[:], in_=seg(Q_W1X, 4096, FH))
            dma(out=w1zT_r[:], in_=seg(Q_W1Z, 8192, FH))
            dma(out=b1_r[:], in_=seg(Q_B1, 64, 1))
            for h in range(2):
                cs = slice(64 * h, 64 * h + 64)
                dma(out=w2T_zd[:, cs], in_=seg(Q_W2Z, 4096, NF))
                dma(out=w2T_gd[:, cs], in_=seg(Q_W2G, 4096, NF))
                dma(out=wsxT_zd_r[:, cs], in_=seg(Q_WSXZ, 4096, NF))
                dma(out=wsxT_gd_r[:, cs], in_=seg(Q_WSXG, 4096, NF))
                dma(out=wszT_zd_r[:, cs], in_=seg(Q_WSZZ, 8192, NF))
                dma(out=wszT_gd_r[:, cs], in_=seg(Q_WSZG, 8192, NF))
                ps_ = slice(64 * h, 64 * h + 64)
                dma(out=b2z_d[ps_], in_=seg(Q_B2, 64, 1))
                dma(out=b2g_d[ps_], in_=seg(Q_B2 + 64, 64, 1))
                dma(out=bsz_d[ps_], in_=seg(Q_BS, 64, 1))
                dma(out=bsg_d[ps_], in_=seg(Q_BS + 64, 64, 1))
            nc.vector.tensor_add(out=bzg_zd[:], in0=b2z_d[:], in1=bsz_d[:])
            nc.vector.tensor_add(out=bzg_gd[:], in0=b2g_d[:], in1=bsg_d[:])
            # prebake +-1/8 (sx rows +, sy rows -) into sobel input rows
            nc.vector.memset(pm_s[0:64], 0.125)
            nc.vector.memset(pm_s[64:128], -0.125)
            nc.vector.tensor_scalar_mul(out=w1zT_r[:], in0=w1zT_r[:],
                                        scalar1=pm_s[:])
            nc.vector.tensor_scalar_mul(out=wszT_zd_r[:], in0=wszT_zd_r[:],
                                        scalar1=pm_s[:])
            nc.vector.tensor_scalar_mul(out=wszT_gd_r[:], in0=wszT_gd_r[:],
                                        scalar1=pm_s[:])
            nc.vector.memset(eps_s[:], EPS)
            dma(out=lf_s[:], in_=lfv[:, :])
            dma(out=seedc_s[:], in_=seedc[:, :])
            dma(out=r1T_s[:], in_=r1T[:, :])
            dma(out=r1b_s[:], in_=r1b[:, :])
            dma(out=r2T_s[:], in_=r2T[:, :])
            dma(out=r2b_s[:], in_=r2b[:, :])
            dma(out=ocT_s[:], in_=ocT[:, :])
            dma(out=ocb_s[:], in_=ocb[:, :])

            # ---------------- Phase 3: init state (blur ca + seed) ---------
            nc.vector.memset(xp2[:, 0:18 * 18], 0.0)
            nc.vector.memset(cas[:], 0.0)
            nc.vector.memset(tcol[:], 0.0)
            dma(out=cas[:, 1:17, 1:17], in_=ca.rearrange("c (h w) -> c h w", w=16))
            # vertical gauss (zero pad): rows of cas
            _axis3tap(nc, tcol[:, :, 1:17],
                      cas[:, 0:16, 1:17], cas[:, 1:17, 1:17], cas[:, 2:18, 1:17],
                      G0, G1, G0)
            xp2v16 = xp2[:, 0:324].rearrange("p (h w) -> p h w", w=18)
            x0i = xp2v16[0:64, 1:17, 1:17]
            _axis3tap(nc, x0i,
                      tcol[:, :, 0:16], tcol[:, :, 1:17], tcol[:, :, 2:18],
                      G0, G1, G0)
            nc.vector.tensor_scalar_add(out=x0i, in0=x0i, scalar1=seedc_s[:])
            dma(out=xp2[64:128, 0:324], in_=xp2[0:64, 0:324])
            nc.vector.tensor_scalar_mul(out=xp2[64:128, 0:324],
                                        in0=xp2[64:128, 0:324], scalar1=-1.0)

            # ---------------- Phase 4: CA iterations -----------------------
            work = ctx.enter_context(tc.tile_pool(name="work", bufs=3))
            pp = ctx.enter_context(tc.tile_pool(name="mainps", bufs=2, space="PSUM"))

            for (R, ncalls) in RES_PLAN:
                H = W = R
                S = R * R
                Wp = R + 2
                PADSZ = (R + 2) * (R + 2)
                xp2v = xp2[:, 0:PADSZ].rearrange("p (h w) -> p h w", w=Wp)
                zAv = zA[:, 0:S].rearrange("p (h w) -> p h w", w=W)
                srows = STRIP_ROWS[R]
                abv = abP[:, 0:srows * Wp].rearrange("p (h w) -> p h w", w=Wp)
                nc.vector.memset(abP[:, 0:srows * Wp], 0.0)
                trows = TILE_ROWS[R]
                ntiles = H // trows

                for call in range(ncalls):
                    # ---- sobel strips: a (lo), b' (hi) -> sx, sy' into zA
                    for s0 in range(0, H, srows):
                        up = xp2v[:, s0:s0 + srows, 1:W + 1]
                        ce = xp2v[:, s0 + 1:s0 + 1 + srows, 1:W + 1]
                        dn = xp2v[:, s0 + 2:s0 + 2 + srows, 1:W + 1]
                        lo = slice(0, 64)
                        hi = slice(64, 128)
                        ai = abv[lo, :, 1:W + 1]
                        bi = abv[hi, :, 1:W + 1]
                        # a = x_up + x_dn + 2x   (lower: +x)
                        nc.vector.tensor_add(out=ai, in0=up[lo], in1=dn[lo])
                        nc.vector.scalar_tensor_tensor(
                            out=ai, in0=ce[lo], scalar=2.0, in1=ai,
                            op0=ALU.mult, op1=ALU.add)
                        # b' = x_up - x_dn = xn_dn - xn_up  (upper: -x)
                        nc.vector.tensor_sub(out=bi, in0=dn[hi], in1=up[hi])
                        # sx = a[c+1] - a[c-1]
                        nc.vector.tensor_sub(out=zAv[lo, s0:s0 + srows, :],
                                             in0=abv[lo, :, 2:W + 2],
                                             in1=abv[lo, :, 0:W])
                        # sy' = b'[c-1] + 2 b'[c] + b'[c+1]
                        nc.vector.tensor_add(out=zAv[hi, s0:s0 + srows, :],
                                             in0=abv[hi, :, 0:W],
                                             in1=abv[hi, :, 2:W + 2])
                        nc.vector.scalar_tensor_tensor(
                            out=zAv[hi, s0:s0 + srows, :],
                            in0=abv[hi, :, 1:W + 1], scalar=2.0,
                            in1=zAv[hi, s0:s0 + srows, :],
                            op0=ALU.mult, op1=ALU.add)

                    # ---- stats (bn_stats = one <=512-elem window per call)
                    nz = (S + 511) // 512
                    for ci in range(nz):
                        e0 = ci * 512
                        e1 = min(S, e0 + 512)
                        nc.vector.bn_stats(out=statz[:, ci, :],
                                           in_=zA[:, e0:e1])
                    nc.vector.bn_aggr(out=mvz[:], in_=statz[:, 0:nz, :])
                    # x stats over the whole padded buffer (zeros included),
                    # corrected analytically by the pad ratio below
                    nx = (PADSZ + 511) // 512
                    for ci in range(nx):
                        e0 = ci * 512
                        e1 = min(PADSZ, e0 + 512)
                        nc.vector.bn_stats(out=statx[:, ci, :],
                                           in_=xp2[:, e0:e1])
                    nc.vector.bn_aggr(out=mvx[:], in_=statx[:, 0:nx, :])
                    cr = float(PADSZ) / float(S)
                    # m = m' * cr ; var = (v' + m'^2) * cr - m^2
                    nc.vector.tensor_scalar_mul(out=mx_c[:], in0=mvx[:, 0:1],
                                                scalar1=cr)
                    nc.vector.tensor_mul(out=vx_t[:], in0=mvx[:, 0:1],
                                         in1=mvx[:, 0:1])
                    nc.vector.tensor_add(out=vx_u[:], in0=mvx[:, 1:2],
                                         in1=vx_t[:])
                    nc.vector.tensor_mul(out=vx_t[:], in0=mx_c[:], in1=mx_c[:])
                    nc.vector.scalar_tensor_tensor(out=vx_u[:], in0=vx_u[:],
                                                   scalar=cr, in1=vx_t[:],
                                                   op0=ALU.mult,
                                                   op1=ALU.subtract)
                    nc.scalar.activation(out=tmp1[:], in_=mvz[:, 1:2],
                                         func=AF.Sqrt, bias=eps_s[:],
                                         scale=1.0 / 64.0)
                    nc.vector.reciprocal(out=rstdz[:], in_=tmp1[:])
                    nc.scalar.activation(out=tmp2[:], in_=vx_u[:],
                                         func=AF.Sqrt, bias=eps_s[:], scale=1.0)
                    nc.vector.reciprocal(out=rstdx[:], in_=tmp2[:])
                    nc.vector.tensor_scalar_mul(out=w1zT_f[:], in0=w1zT_r[:],
                                                scalar1=rstdz[:])
                    nc.vector.tensor_scalar_mul(out=w1xT_f[:], in0=w1xT_r[:],
                                                scalar1=rstdx[0:64])
                    nc.vector.tensor_scalar_mul(out=wszT_zf[:], in0=wszT_zd_r[:],
                                                scalar1=rstdz[:])
                    nc.vector.tensor_scalar_mul(out=wszT_gf[:], in0=wszT_gd_r[:],
                                                scalar1=rstdz[:])
                    nc.vector.tensor_scalar_mul(out=wsxT_zf[:], in0=wsxT_zd_r[:],
                                                scalar1=rstdx[0:64])
                    nc.vector.tensor_scalar_mul(out=wsxT_gf[:], in0=wsxT_gd_r[:],
                                                scalar1=rstdx[0:64])
                    # bias folds: b' = b_raw - W_f @ m
                    psb1_t = pp.tile([128, 1], DT, tag="psb")
                    psb1 = psb1_t[0:NF]
                    nc.tensor.matmul(psb1[:], w1zT_f[:], mvz[:, 0:1],
                                     start=True, stop=False)
                    nc.tensor.matmul(psb1[:], w1xT_f[:], mx_c[0:64],
                                     start=False, stop=True)
                    nc.vector.tensor_sub(out=b1_f[:], in0=b1_r[:], in1=psb1[:])
                    psbz = pp.tile([128, 1], DT, tag="psb")
                    nc.tensor.matmul(psbz[:], wszT_zf[:], mvz[:, 0:1],
                                     start=True, stop=False)
                    nc.tensor.matmul(psbz[:], wsxT_zf[:], mx_c[0:64],
                                     start=False, stop=True)
                    nc.vector.tensor_sub(out=bz_f[:], in0=bzg_zd[:], in1=psbz[:])
                    psbg = pp.tile([128, 1], DT, tag="psb")
                    nc.tensor.matmul(psbg[:], wszT_gf[:], mvz[:, 0:1],
                                     start=True, stop=False)
                    nc.tensor.matmul(psbg[:], wsxT_gf[:], mx_c[0:64],
                                     start=False, stop=True)
                    nc.vector.tensor_sub(out=bg_f[:], in0=bzg_gd[:], in1=psbg[:])

                    # ---- dyna block per pixel tile
                    for t in range(ntiles):
                        r0 = t * trows
                        zsl = zAv[:, r0:r0 + trows, :]
                        xw = xp2v[0:64, r0 + 1:r0 + 1 + trows, 1:W + 1]
                        xw128 = xp2v[:, r0 + 1:r0 + 1 + trows, 1:W + 1]
                        psh = pp.tile([NF, trows, W], DT, tag="psh")
                        nc.tensor.matmul(psh[:], w1zT_f[:], zsl,
                                         start=True, stop=False)
                        nc.tensor.matmul(psh[:], w1xT_f[:], xw,
                                         start=False, stop=True)
                        hs = work.tile([NF, trows, W], DT, tag="hs")
                        nc.scalar.activation(out=hs[:], in_=psh[:], func=AF.Relu,
                                             bias=b1_f[:], scale=1.0)
                        psz = pp.tile([128, trows, W], DT, tag="psz")
                        nc.tensor.matmul(psz[:], w2T_zd[:], hs[:],
                                         start=True, stop=False)
                        nc.tensor.matmul(psz[:], wszT_zf[:], zsl,
                                         start=False, stop=False)
                        nc.tensor.matmul(psz[:], wsxT_zf[:], xw,
                                         start=False, stop=True)
                        psg = pp.tile([128, trows, W], DT, tag="psg")
                        nc.tensor.matmul(psg[:], w2T_gd[:], hs[:],
                                         start=True, stop=False)
                        nc.tensor.matmul(psg[:], wszT_gf[:], zsl,
                                         start=False, stop=False)
                        nc.tensor.matmul(psg[:], wsxT_gf[:], xw,
                                         start=False, stop=True)
                        sig = work.tile([128, trows, W], DT, tag="sig")
                        nc.scalar.activation(out=sig[:], in_=psg[:],
                                             func=AF.Sigmoid, bias=bg_f[:],
                                             scale=1.0)
                        zg = work.tile([128, trows, W], DT, tag="zg")
                        nc.vector.scalar_tensor_tensor(
                            out=zg[:], in0=psz[:], scalar=bz_f[:], in1=sig[:],
                            op0=ALU.add, op1=ALU.mult)
                        nc.vector.scalar_tensor_tensor(
                            out=xw128, in0=zg[:], scalar=lf_s[:], in1=xw128,
                            op0=ALU.mult, op1=ALU.add)

                    # ---- resolution transition
                    if call == ncalls - 1 and R < IMG:
                        H2, W2 = 2 * H, 2 * W
                        PADSZ2 = (H2 + 2) * (W2 + 2)
                        # uv staging in zA lower: [64, H, 2, W] (rows 2j+q)
                        uv = zA[0:64, 0:H2 * W].rearrange(
                            "p (h q w) -> p h q w", q=2, w=W)
                        xr = xp2v[0:64, :, 1:W + 1]  # padded rows, img cols
                        # vertical: even parity q=0, odd q=1
                        for q, (cl, cc, ch) in enumerate(
                                [(A_E, B_E, C_E), (C_E, B_E, A_E)]):
                            _axis3tap(nc, uv[:, 1:H - 1, q, :],
                                      xr[:, 1:H - 1, :], xr[:, 2:H, :],
                                      xr[:, 3:H + 1, :], cl, cc, ch)
                            # j=0 edge (clamp; q=0 also blur-edge corr)
                            c0 = cl + cc - (G0 if q == 0 else 0.0)
                            _axis2tap(nc, uv[:, 0:1, q, :],
                                      xr[:, 1:2, :], xr[:, 2:3, :], c0, ch)
                            # j=H-1 edge (clamp; q=1 blur-edge corr)
                            cH = cc + ch - (G0 if q == 1 else 0.0)
                            _axis2tap(nc, uv[:, H - 1:H, q, :],
                                      xr[:, H:H + 1, :], xr[:, H - 1:H, :],
                                      cH, cl)
                        # clear new-geometry lower region, then horizontal pass
                        nc.vector.memset(xp2[0:64, 0:PADSZ2], 0.0)
                        xpq = xp2[:, 0:PADSZ2].rearrange(
                            "p (h c q) -> p h c q", c=(W2 + 2) // 2, q=2)
                        uvf = zA[0:64, 0:H2 * W].rearrange(
                            "p (h w) -> p h w", w=W)
                        for q, (cl, cc, ch) in enumerate(
                                [(A_E, B_E, C_E), (C_E, B_E, A_E)]):
                            if q == 0:  # even out col c_out -> padded 1+2c: q=1
                                oi = xpq[0:64, 1:H2 + 1, 1:W - 1, 1]
                                o0 = xpq[0:64, 1:H2 + 1, 0:1, 1]
                                oL = xpq[0:64, 1:H2 + 1, W - 1:W, 1]
                            else:       # odd out col -> padded 2+2c: q=0,c+1
                                oi = xpq[0:64, 1:H2 + 1, 2:W, 0]
                                o0 = xpq[0:64, 1:H2 + 1, 1:2, 0]
                                oL = xpq[0:64, 1:H2 + 1, W:W + 1, 0]
                            _axis3tap(nc, oi,
                                      uvf[:, :, 0:W - 2], uvf[:, :, 1:W - 1],
                                      uvf[:, :, 2:W], cl, cc, ch)
                            c0 = cl + cc - (G0 if q == 0 else 0.0)
                            _axis2tap(nc, o0, uvf[:, :, 0:1], uvf[:, :, 1:2],
                                      c0, ch)
                            cL = cc + ch - (G0 if q == 1 else 0.0)
                            _axis2tap(nc, oL, uvf[:, :, W - 1:W],
                                      uvf[:, :, W - 2:W - 1], cL, cl)
                        dma(out=xp2[64:128, 0:PADSZ2], in_=xp2[0:64, 0:PADSZ2])
                        nc.vector.tensor_scalar_mul(out=xp2[64:128, 0:PADSZ2],
                                                    in0=xp2[64:128, 0:PADSZ2],
                                                    scalar1=-1.0)

            # ---------------- Phase 5: output head -------------------------
            R = IMG
            W = R
            Wp = R + 2
            xp2v = xp2[:, 0:Wp * Wp].rearrange("p (h w) -> p h w", w=Wp)
            trows = TILE_ROWS[R]
            for t in range(R // trows):
                r0 = t * trows
                xw = xp2v[0:64, r0 + 1:r0 + 1 + trows, 1:W + 1]
                ps1 = pp.tile([NF, trows, W], DT, tag="psh")
                nc.tensor.matmul(ps1[:], r1T_s[:], xw, start=True, stop=True)
                h1 = work.tile([NF, trows, W], DT, tag="hs")
                nc.scalar.activation(out=h1[:], in_=ps1[:], func=AF.Relu,
                                     bias=r1b_s[:], scale=1.0)
                ps2 = pp.tile([NF, trows, W], DT, tag="psz")
                nc.tensor.matmul(ps2[:], r2T_s[:], h1[:], start=True, stop=True)
                xr_t = work.tile([NF, trows, W], DT, tag="sig")
                nc.vector.scalar_tensor_tensor(out=xr_t[:], in0=ps2[:],
                                               scalar=r2b_s[:], in1=xw,
                                               op0=ALU.add, op1=ALU.add)
                ps3 = pp.tile([3, trows, W], DT, tag="psg")
                nc.tensor.matmul(ps3[:], ocT_s[:], xr_t[:], start=True, stop=True)
                yr = work.tile([3, trows, W], DT, tag="yr")
                nc.vector.tensor_scalar_add(out=yr[:], in0=ps3[:],
                                            scalar1=ocb_s[:])
                yc = work.tile([3, trows, W], DT, tag="yc")
                nc.vector.tensor_scalar(out=yc[:], in0=yr[:], scalar1=-1.0,
                                        scalar2=1.0, op0=ALU.max, op1=ALU.min)
                sl = slice(r0 * W, (r0 + trows) * W)
                dma(out=y_raw[:, sl].rearrange("p (a b) -> p a b", b=W), in_=yr[:])
                dma(out=y_clip[:, sl].rearrange("p (a b) -> p a b", b=W), in_=yc[:])


    nc.finalize()
    return nc


_CACHE = {}


def prep_in_maps(lat, ca_noise, seed, leak_factor, hyper_w, hyper_b,
                 res1_w, res1_b, res2_w, res2_b, outc_w, outc_b):
    f32 = lambda a: np.ascontiguousarray(np.asarray(a, np.float32))
    lat, ca_noise = f32(lat), f32(ca_noise)
    hyper_w, hyper_b = f32(hyper_w), f32(hyper_b)
    if "perm" not in _CACHE:
        _CACHE["perm"] = build_perm()
    perm = _CACHE["perm"]

    hw_p = np.ascontiguousarray(hyper_w[:, perm])
    hb_p = np.ascontiguousarray(hyper_b[perm])
    latT = np.ascontiguousarray(lat.T)
    lf = np.float32(np.clip(np.asarray(leak_factor, np.float32), 0.001, 1000.0))
    lfvec = np.concatenate([np.full((64, 1), lf, np.float32),
                            np.full((64, 1), -lf, np.float32)])
    seedc = seed_branch_consts(seed)
    common = dict(
        latT=latT, lfv=lfvec, seedc=seedc,
        r1T=f32(res1_w).T.copy(), r1b=f32(res1_b).reshape(NF, 1),
        r2T=f32(res2_w).T.copy(), r2b=f32(res2_b).reshape(NF, 1),
        ocT=f32(outc_w).T.copy(), ocb=f32(outc_b).reshape(3, 1),
    )
    in_maps = []
    for c in range(N_CORES):
        m = dict(common)
        m["hw"] = np.ascontiguousarray(hw_p[:, c * CHUNK:(c + 1) * CHUNK])
        m["hb"] = np.ascontiguousarray(hb_p[None, c * CHUNK:(c + 1) * CHUNK])
        m["ca"] = np.ascontiguousarray(ca_noise[c].reshape(NF, 256))
        in_maps.append(m)
    return in_maps


def kernel(**inputs):
    if "nc" not in _CACHE:
        _CACHE["nc"] = build_nc()
    nc = _CACHE["nc"]
    in_maps = prep_in_maps(**inputs)
    res = run_bass_kernel_spmd(nc, in_maps, core_ids=list(range(N_CORES)),
                               **_CACHE.get("run_kwargs", {}))
    _CACHE["last_res"] = res
    clip = np.stack([res.results[c]["y_clip"].reshape(3, IMG, IMG)
                     for c in range(N_CORES)])
    raw = np.stack([res.results[c]["y_raw"].reshape(3, IMG, IMG)
                    for c in range(N_CORES)])
    return clip, raw

